# revision 14
# baseline (speedup 1.0000x reference)
"""Trainium2 Bass kernel for the histogram-binning KL loss.

Strategy (v3)
-------------
 * Matrix-parallel sharding: cores 0-3 process T_F, cores 4-7 process S_F,
   64 cosine rows (16384 pairs) per core.  SPMD-identical program; only the
   per-core input tensors differ.
 * Gaussian soft-binning on a 55-point coarse grid (20x decimation of the
   1000 fine bins, exact division); full histograms recovered by 6-point
   Lagrange interpolation.  The coarse grid spacing 0.04 makes 100*t exactly
   representable in bf16, so the q matmul needs no split-precision row for
   the t-table: q = (100 t)(dh+dl) - 50(sqh+sql) + (lwh+lwl), KQ=8 rows.
   The four log-weight rows are host-prestitched into the rhs layout and
   DMA'd directly from DRAM; only 4 rows (dh/dl/sqh/sql) are stitched
   on-device.
 * ScalarE evaluates exp(q - 50 t^2) via its per-partition bias, 2048 pairs
   per pass, and its fused accum register IS the weighted histogram partial.
 * The NRT AllReduce is replaced by a peer-to-peer remote-DMA all-gather
   (SBUF -> SBUF, gpsimd SWDGE descriptors + remote semaphores): each core
   broadcasts its [128 x 11] f32 payload slot-wise to all 8 cores in two
   phases (E rows + o3 early, hidden under the exp loop; the 2-column
   histogram partials late), then sums the 8 slots locally with 3 vector
   adds.  No DRAM round-trip, no NRT mesh latency.

Host work is limited to argmax/label-mask construction and constant tables.
"""

import os
from contextlib import ExitStack

import ml_dtypes
import numpy as np

import concourse.bass as bass
import concourse.bacc as bacc
import concourse.tile as tile
from concourse import masks, mybir
from concourse.bass_utils import run_bass_kernel_spmd

F32 = mybir.dt.float32
BF16 = mybir.dt.bfloat16
NPBF = ml_dtypes.bfloat16
AF = mybir.ActivationFunctionType

N, D, C = 256, 512, 16
N_CORES = 8
GCORES = 4                     # cores per matrix
ROWS = N // GCORES             # 64 cosine rows per core
PAIRS = ROWS * N               # 16384 pair distances per core
S = 20                         # fine bins per coarse bin (exact: 1000/20)
ORDER = 6                      # Lagrange interpolation order
MC = 1000 // S + ORDER - 1     # 55 coarse bins
HALF = 64                      # partition half (pos bins 0:55, neg 64:119)
KQ = 8                         # contraction rows of the q matmul
BLK = 512                      # pairs per matmul (one PSUM bank)
GRP = 2048                     # pairs per exp pass (4 blocks)
NGRP = PAIRS // GRP            # 8
NB = 1000
NBP = 1024                     # padded fine bins (zero tail)
EPS = 1e-9
INV2S2 = 50.0                  # 1 / (2 sigma^2)
LOG_ZERO = -60000.0            # ln(0) stand-in; exp underflows to exactly 0

PL = 11                        # payload cols: histT histS E(8: k*2+hi) o3
NSL = 8                        # gather slots

# mega const f32 [128, MEGA_COLS] column layout
MG_MP = 0          # [64, 256]
MG_MN = 256        # [64, 256]
MG_PLO = 512       # Pm2_lo [64, 128]
MG_PHI = 640       # Pm2_hi [64, 128]
MG_M2 = 768        # [128, 2]
MG_BQ = 770        # [128, 1]
MG_KC = 771        # [2, 1]
MEGA_COLS = 772


def _bfsplit(x, n=2):
    """Split x into n bf16 terms summing to ~x (exact bf16 values)."""
    out, r = [], np.asarray(x, np.float64)
    for _ in range(n):
        h = r.astype(NPBF)
        out.append(h)
        r = r - h.astype(np.float64)
    return out


def _coarse_centers():
    m = np.arange(HALF, dtype=np.float64)
    return -1.0 + (0.002 * S) * (m - 1.0)   # entries >= MC are padding


def _rq_table():
    t100 = 2 * INV2S2 * _coarse_centers()   # integers: exact in bf16
    t100[MC:] = 0.0
    th = np.concatenate([t100, t100])
    indp = np.zeros(2 * HALF)
    indp[:MC] = 1.0
    indn = np.zeros(2 * HALF)
    indn[HALF : HALF + MC] = 1.0
    m50 = -INV2S2 * (indp + indn)
    # row k pairs with st row k: [dh dl sqh sql lwph lwpl lwnh lwnl]
    return np.stack([th, th, m50, m50, indp, indp, indn, indn]).astype(NPBF)


def _bq_table():
    t = _coarse_centers()
    bq = np.concatenate([-INV2S2 * t * t, -INV2S2 * t * t])[:, None]
    bq[MC:HALF] = LOG_ZERO
    bq[HALF + MC :] = LOG_ZERO
    return bq.astype(np.float32)


def _interp_table():
    wi = np.zeros((HALF, NBP), np.float64)
    nodes = np.arange(ORDER) - 1.0
    for r in range(S):
        x = r / S
        c = [
            np.prod([(x - nodes[j]) / (nodes[m] - nodes[j]) for j in range(ORDER) if j != m])
            for m in range(ORDER)
        ]
        ks = np.arange((NB - r + S - 1) // S)
        for m in range(ORDER):
            wi[ks + m, S * ks + r] = c[m]
    return np.vstack([wi, wi]).astype(np.float32)   # [128, NBP]


def build_nc():
    nc = bacc.Bacc(
        "TRN2", target_bir_lowering=False, debug=False, num_devices=N_CORES
    )

    xd = nc.dram_tensor("x", [N, D], F32, kind="ExternalInput")
    xrd = nc.dram_tensor("xr", [ROWS, D], F32, kind="ExternalInput")
    MGd = nc.dram_tensor("MG", [128, MEGA_COLS], F32, kind="ExternalInput")
    LWd = nc.dram_tensor("LW", [4, PAIRS], BF16, kind="ExternalInput")
    Rqd = nc.dram_tensor("Rq", [KQ, 2 * HALF], BF16, kind="ExternalInput")
    WId = nc.dram_tensor("WI", [2 * HALF, NBP], F32, kind="ExternalInput")
    outd = nc.dram_tensor("out", [1, 1], F32, kind="ExternalOutput")
    dbgd = nc.dram_tensor("dbg", [128, (NSL + 2) * PL], F32, kind="ExternalOutput")

    rsem = nc.alloc_semaphore("rdma_recv")
    lsem = nc.alloc_semaphore("rdma_sent")

    with tile.TileContext(nc) as tc, ExitStack() as ctx:
        cpool = ctx.enter_context(tc.tile_pool(name="const", bufs=1))
        spool = ctx.enter_context(tc.tile_pool(name="stitch", bufs=1))
        xpool = ctx.enter_context(tc.tile_pool(name="x", bufs=1))
        tpool = ctx.enter_context(tc.tile_pool(name="xnt", bufs=2))
        mpool = ctx.enter_context(tc.tile_pool(name="misc", bufs=2))
        rpool = ctx.enter_context(tc.tile_pool(name="res", bufs=1))

        # ---- input DMAs: x gates the front-end; issue on sync + scalar
        xa = xpool.tile([128, 2 * D], F32)          # [p, (h d)]
        nc.sync.dma_start(
            xa[:].rearrange("p (h d) -> p h d", h=2),
            xd[:, :].rearrange("(h p) d -> p h d", p=128),
        )
        xra = xpool.tile([ROWS, D], F32)
        nc.sync.dma_start(xra[:], xrd[:, :])
        MG = cpool.tile([128, MEGA_COLS], F32)
        nc.scalar.dma_start(MG[:], MGd[:, :])
        st = spool.tile([KQ, PAIRS], BF16)
        nc.sync.dma_start(st[4:8, :], LWd[:, :])
        Rq = cpool.tile([KQ, 2 * HALF], BF16)
        nc.scalar.dma_start(Rq[:], Rqd[:, :])
        WI = cpool.tile([2 * HALF, NBP], F32)
        nc.scalar.dma_start(WI[:], WId[:, :])

        ident = cpool.tile([128, 128], F32)
        masks.make_identity(nc, ident[:])
        ones = cpool.tile([128, 1], F32)
        nc.vector.memset(ones[:], 1.0)

        payload = rpool.tile([128, PL], F32)
        nc.vector.memset(payload[:], 0.0)
        gather = rpool.tile([128, NSL * PL], F32)
        nc.vector.memset(gather[:], 0.0)

        # Generate all remote-DMA descriptors up front (address-only work on
        # gpsimd, hidden under the front-end).  Slot-wise XOR addressing:
        # broadcast j sends this core's payload to peer (tpb XOR j), landing
        # in gather slot j on the receiver, so slot j holds peer (self XOR j)
        # and every slot holds a distinct core.  The transfers only fire at
        # the trigger_dma below, after all payload writes.
        for j in range(NSL):
            rd = [None] * NSL
            rd[j] = (0, j)
            nc.gpsimd.remote_dma_broadcast(
                gather[:, PL * j : PL * (j + 1)],
                payload[:, :],
                rsem, lsem, rdests=rd,
            )

        # ---- row-normalize the full matrix and this core's slice
        junk = xpool.tile([128, 2 * D], F32, tag="junk")
        nrm2 = mpool.tile([128, 2], F32, tag="nrm2")
        for h in range(2):
            sl = slice(D * h, D * (h + 1))
            nc.vector.scalar_tensor_tensor(
                junk[:, sl], xa[:, sl], 1.0, xa[:, sl],
                mybir.AluOpType.bypass, mybir.AluOpType.mult,
                accum_out=nrm2[:, h : h + 1],
            )
        srt = mpool.tile([128, 2], F32, tag="srt")
        nc.scalar.activation(srt[:], nrm2[:], AF.Sqrt)
        rn = mpool.tile([128, 2], F32, tag="rn")
        nc.vector.reciprocal(rn[:], srt[:])
        xn = xpool.tile([128, 2 * D], F32, tag="xn")
        for h in range(2):
            sl = slice(D * h, D * (h + 1))
            nc.vector.tensor_scalar_mul(xn[:, sl], xa[:, sl], rn[:, h : h + 1])

        junkr = xpool.tile([ROWS, D], F32, tag="junkr")
        nrm2r = mpool.tile([ROWS, 1], F32, tag="nrm2r")
        nc.vector.scalar_tensor_tensor(
            junkr[:], xra[:], 1.0, xra[:],
            mybir.AluOpType.bypass, mybir.AluOpType.mult,
            accum_out=nrm2r[:],
        )
        srtr = mpool.tile([ROWS, 1], F32, tag="srtr")
        nc.scalar.activation(srtr[:], nrm2r[:], AF.Sqrt)
        rnr = mpool.tile([ROWS, 1], F32, tag="rnr")
        nc.vector.reciprocal(rnr[:], srtr[:])
        xnr = xpool.tile([ROWS, D], F32, tag="xnr")
        nc.vector.tensor_scalar_mul(xnr[:], xra[:], rnr[:])

        # ---- transpose into d-major layout, accumulate the cos Gram slice,
        #      then the E columns + o3 partial (stage-A PSUM pool)
        with tc.tile_pool(name="psA", bufs=2, space="PSUM") as ppA:
            cps = ppA.tile([ROWS, N], F32, tag="cos_ps", bufs=1)
            for c in range(4):
                xt = tpool.tile([128, N], F32, tag="xnT")
                for h in range(2):
                    pt = ppA.tile([128, 128], F32, tag="ps_t")
                    nc.tensor.transpose(
                        pt[:], xn[:, D * h + 128 * c : D * h + 128 * (c + 1)],
                        ident[:],
                    )
                    nc.vector.tensor_copy(xt[:, 128 * h : 128 * (h + 1)], pt[:])
                ptr = ppA.tile([128, ROWS], F32, tag="ps_t")
                nc.tensor.transpose(
                    ptr[:], xnr[:, 128 * c : 128 * (c + 1)], ident[:ROWS, :ROWS]
                )
                xtr = tpool.tile([128, ROWS], F32, tag="xnrT")
                nc.vector.tensor_copy(xtr[:], ptr[:])
                nc.tensor.matmul(
                    cps[:], xtr[:], xt[:], start=(c == 0), stop=(c == 3)
                )
            cos_sb = mpool.tile([ROWS, N], F32, tag="cos_sb")
            nc.vector.tensor_copy(cos_sb[:], cps[:])

            # E columns (weighted row means of cos) + o3 partial
            e2 = rpool.tile([ROWS, 2], F32)
            junkE = mpool.tile([ROWS, N], F32, tag="junkE")
            for col, c0 in ((0, MG_MP), (1, MG_MN)):
                nc.vector.scalar_tensor_tensor(
                    junkE[:], cos_sb[:], 1.0, MG[0:ROWS, c0 : c0 + N],
                    mybir.AluOpType.bypass, mybir.AluOpType.mult,
                    accum_out=e2[:, col : col + 1],
                )
            ed = rpool.tile([ROWS, 1], F32)
            nc.vector.tensor_sub(ed[:], e2[:, 0:1], e2[:, 1:2])
            o3_ps = ppA.tile([1, 1], F32, tag="ps_s", bufs=1)
            nc.tensor.matmul(
                o3_ps[:], MG[0:ROWS, MG_M2 + 1 : MG_M2 + 2], ed[:],
                start=True, stop=True,
            )
            nc.vector.tensor_copy(payload[0:1, 10:11], o3_ps[:])
            # mask E by matrix (cols: EposT EnegT EposS EnegS), scatter the
            # 64 rows into [128] partitions, split lo/hi halves by matmul
            e4 = rpool.tile([ROWS, 4], F32)
            nc.vector.tensor_scalar_mul(
                e4[:, 0:2], e2[:], MG[0:ROWS, MG_M2 : MG_M2 + 1]
            )
            nc.vector.tensor_scalar_mul(
                e4[:, 2:4], e2[:], MG[0:ROWS, MG_M2 + 1 : MG_M2 + 2]
            )
            ep_lo = ppA.tile([128, 4], F32, tag="ep", bufs=2)
            nc.tensor.matmul(
                ep_lo[:], MG[0:ROWS, MG_PLO : MG_PLO + 128], e4[:],
                start=True, stop=True,
            )
            ep_hi = ppA.tile([128, 4], F32, tag="ep", bufs=2)
            nc.tensor.matmul(
                ep_hi[:], MG[0:ROWS, MG_PHI : MG_PHI + 128], e4[:],
                start=True, stop=True,
            )
            nc.vector.tensor_copy(payload[:, 2:10:2], ep_lo[:])
            nc.vector.tensor_copy(payload[:, 3:10:2], ep_hi[:])

            # ---- split-bf16 component blocks, stitched into st rows 0-3
            sb4 = spool.tile([ROWS, 4 * N], BF16)

            def blk_(r):
                return sb4[:, N * r : N * (r + 1)]

            sq_sb = mpool.tile([ROWS, N], F32, tag="sq_sb")
            nc.vector.tensor_mul(sq_sb[:], cos_sb[:], cos_sb[:])
            nc.scalar.copy(blk_(0), cos_sb[:])                 # dh
            nc.vector.tensor_sub(blk_(1), cos_sb[:], blk_(0))  # dl
            nc.scalar.copy(blk_(2), sq_sb[:])                  # sqh
            nc.vector.tensor_sub(blk_(3), sq_sb[:], blk_(2))   # sql

            for r, eng in ((0, nc.sync), (1, nc.scalar), (2, nc.sync), (3, nc.scalar)):
                eng.dma_start(
                    st[r : r + 1, :].rearrange("p (r c) -> p r c", r=ROWS),
                    blk_(r),
                )

        # ---- main loop: q matmul -> exp with fused histogram accum
        hacc = rpool.tile([128, NGRP], F32)
        djunk = mpool.tile([128, GRP], BF16, tag="djunk", bufs=1)
        with tc.tile_pool(name="psB", bufs=2, space="PSUM") as ppB:
            for g in range(NGRP):
                q2 = ppB.tile([128, GRP], F32, tag="q2")
                for b in range(GRP // BLK):
                    lo = GRP * g + BLK * b
                    nc.tensor.matmul(
                        q2[:, BLK * b : BLK * (b + 1)],
                        Rq[:],
                        st[:, lo : lo + BLK],
                        start=True,
                        stop=True,
                    )
                nc.scalar.activation(
                    djunk[:], q2[:], AF.Exp,
                    bias=MG[:, MG_BQ : MG_BQ + 1],
                    accum_out=hacc[:, g : g + 1],
                )

        # preload the Ln activation table while the exchange is in flight
        junkln = rpool.tile([1, 1], F32)
        nc.scalar.activation(junkln[:], hacc[0:1, 0:1], AF.Ln)

        hcol = rpool.tile([128, 1], F32)
        nc.vector.reduce_sum(hcol[:], hacc[:], axis=mybir.AxisListType.X)
        nc.vector.tensor_scalar_mul(
            payload[:, 0:2], MG[:, MG_M2 : MG_M2 + 2], hcol[:]
        )

        # ---- fire the exchange, wait for all 8 arrivals (2 sem units each),
        # sum the 8 slots.  The critical section guarantees (a) the trigger
        # executes after every descgen and payload write (its entry barrier
        # snapshots the global clock at this program point) and (b) the
        # remote-sem wait -- unsatisfiable in the tile scheduler's
        # single-core sim -- is scheduled as a black box.  The sem_clear
        # resets the arrival count for the next execution; it is ordered
        # after the wait by the DVE instruction stream.
        t1 = rpool.tile([128, 4 * PL], F32)
        t2 = rpool.tile([128, 2 * PL], F32)
        gs = rpool.tile([128, PL], F32)
        pj = rpool.tile([1, 1], F32)
        with tc.tile_critical(name="gathersum"):
            # reading payload here makes it an input tensor of the critical
            # section, so the entry barrier waits for every payload write
            # before any engine (in particular the trigger) proceeds
            nc.vector.tensor_copy(pj[:], payload[0:1, 0:1])
            # the prelude AllGather barrier guarantees every peer has entered
            # the kernel (and passed its preamble sem_clear) before any data
            # is pushed into its SBUF; it also makes NRT treat this NEFF as
            # collective-bearing, which synchronizes core dispatch
            nc.gpsimd.bir_kernel_barrier_wait([list(range(N_CORES))])
            nc.gpsimd.trigger_dma(count=NSL)
            nc.vector.wait_ge(rsem, 2 * NSL)
            nc.vector.tensor_add(t1[:], gather[:, : 4 * PL], gather[:, 4 * PL :])
            nc.vector.tensor_add(t2[:], t1[:, : 2 * PL], t1[:, 2 * PL :])
            nc.vector.tensor_add(gs[:], t2[:, :PL], t2[:, PL:])
            nc.vector.sem_clear(rsem)

        # ---- tail: interpolate fine hists, KL + order terms (stage-C PSUM)
        with tc.tile_pool(name="psC", bufs=1, space="PSUM") as ppC:
            lh4 = rpool.tile([128, 34], F32)
            nc.vector.memset(lh4[:], 0.0)
            nc.vector.tensor_copy(lh4[0:HALF, 0:1], gs[0:HALF, 0:1])
            nc.vector.tensor_copy(lh4[HALF:, 1:2], gs[HALF:, 0:1])
            nc.vector.tensor_copy(lh4[0:HALF, 32:33], gs[0:HALF, 1:2])
            nc.vector.tensor_copy(lh4[HALF:, 33:34], gs[HALF:, 1:2])

            dE = rpool.tile([128, 4], F32)
            nc.vector.tensor_sub(dE[:], gs[:, 2:6], gs[:, 6:10])
            nE = rpool.tile([128, 4], F32)
            nc.vector.tensor_scalar_mul(nE[:], dE[:], -1.0)
            aE = rpool.tile([128, 4], F32)
            nc.vector.tensor_tensor(aE[:], dE[:], nE[:], mybir.AluOpType.max)
            acol = rpool.tile([128, 1], F32)
            nc.vector.reduce_sum(acol[:], aE[:], axis=mybir.AxisListType.X)
            ps3 = ppC.tile([1, 3], F32, tag="ps_s")
            nc.tensor.matmul(ps3[:, 0:1], ones[:], acol[:], start=True, stop=True)

            fh = ppC.tile([128, NBP], F32, tag="fh")
            for half in range(2):
                cols = slice(512 * half, 512 * (half + 1))
                nc.tensor.matmul(
                    fh[0:34, cols], lh4[:], WI[:, cols], start=True, stop=True
                )
            # stack T (cols 0:NBP) and S (cols NBP:) on partitions 0:2 --
            # partition-shifting reads are legal from PSUM, and free-axis
            # slices keep later ops at one base partition
            av = rpool.tile([2, 2 * NBP], F32)
            nc.vector.tensor_scalar(
                av[:, 0:NBP], fh[0:2, :], 0.0, EPS,
                mybir.AluOpType.max, mybir.AluOpType.add,
            )
            nc.vector.tensor_scalar(
                av[:, NBP:], fh[32:34, :], 0.0, EPS,
                mybir.AluOpType.max, mybir.AluOpType.add,
            )
            lnv = rpool.tile([2, 2 * NBP], F32)
            nc.scalar.activation(lnv[:], av[:], AF.Ln)
            dif = rpool.tile([2, NBP], F32)
            nc.vector.tensor_sub(dif[:], lnv[:, 0:NBP], lnv[:, NBP:])
            junkk = rpool.tile([2, NBP], F32)
            kl2 = rpool.tile([2, 1], F32)
            nc.vector.scalar_tensor_tensor(
                junkk[:], av[:, 0:NBP], 1.0, dif[:],
                mybir.AluOpType.bypass, mybir.AluOpType.mult,
                accum_out=kl2[:],
            )
            nc.tensor.matmul(
                ps3[:, 2:3], MG[0:2, MG_KC : MG_KC + 1], kl2[:],
                start=True, stop=True,
            )

            # ---- final scalar: kl + 0.5 * (o12 + o3) / N
            sc3 = rpool.tile([1, 3], F32)
            nc.vector.tensor_copy(sc3[:], ps3[:])
            osum = rpool.tile([1, 1], F32)
            nc.vector.tensor_add(osum[:], sc3[:, 0:1], gs[0:1, 10:11])
            fin = rpool.tile([1, 1], F32)
            nc.vector.scalar_tensor_tensor(
                fin[:], osum[:], 0.5 / N, sc3[:, 2:3],
                mybir.AluOpType.mult, mybir.AluOpType.add,
            )
            nc.sync.dma_start(outd[:, :], fin[:])
            nc.sync.dma_start(dbgd[:, 0 : NSL * PL], gather[:])
            nc.sync.dma_start(dbgd[:, NSL * PL : (NSL + 1) * PL], payload[:])
            nc.sync.dma_start(dbgd[:, (NSL + 1) * PL :], gs[:])

    nc.compile()
    return nc


def _host_inputs(T_F, S_F, labels):
    T_F = np.ascontiguousarray(T_F, np.float32)
    S_F = np.ascontiguousarray(S_F, np.float32)
    labels = np.asarray(labels)
    lab = np.argmax(labels, axis=-1)
    grid = (lab[None, :] == lab[:, None]).astype(np.float32)
    neg_l = 1.0 - grid
    pos_l = grid * (1.0 - np.eye(N, dtype=np.float32))
    pw = pos_l / pos_l.sum()
    nw = neg_l / neg_l.sum()
    lpw = np.full_like(pw, LOG_ZERO, dtype=np.float64)
    np.log(pw, out=lpw, where=pw > 0)
    lnw = np.full_like(nw, LOG_ZERO, dtype=np.float64)
    np.log(nw, out=lnw, where=nw > 0)
    mp = pos_l / pos_l.sum(-1, keepdims=True)
    mn = neg_l / neg_l.sum(-1, keepdims=True)

    rq = _rq_table()
    bq = _bq_table()
    wi = _interp_table()

    in_maps = []
    for c in range(N_CORES):
        is_t = c < GCORES
        mat = T_F if is_t else S_F
        r0 = ROWS * (c % GCORES)
        rows = slice(r0, r0 + ROWS)

        mega = np.zeros((128, MEGA_COLS), np.float32)
        mega[0:ROWS, MG_MP : MG_MP + N] = mp[rows]
        mega[0:ROWS, MG_MN : MG_MN + N] = mn[rows]
        # Pm2_lo/hi: scatter local row -> global row r0+row = hi*128 + b
        plo = np.zeros((ROWS, 128), np.float32)
        phi = np.zeros((ROWS, 128), np.float32)
        tgt = plo if r0 + ROWS <= 128 else phi
        tgt[np.arange(ROWS), (r0 + np.arange(ROWS)) % 128] = 1.0
        mega[0:ROWS, MG_PLO : MG_PLO + 128] = plo
        mega[0:ROWS, MG_PHI : MG_PHI + 128] = phi
        mega[:, MG_M2 + (0 if is_t else 1)] = 1.0
        mega[:, MG_BQ] = bq[:, 0]
        mega[0, MG_KC] = 0.1
        mega[1, MG_KC] = 0.02

        lph, lpl = _bfsplit(lpw[rows].reshape(-1))
        lnh, lnl = _bfsplit(lnw[rows].reshape(-1))
        lw4 = np.stack([lph, lpl, lnh, lnl]).astype(NPBF)

        in_maps.append(
            {
                "x": mat,
                "xr": np.ascontiguousarray(mat[rows]),
                "MG": mega,
                "LW": lw4,
                "Rq": rq,
                "WI": wi,
            }
        )
    return in_maps


_NC_CACHE = {}


def run(T_F, S_F, labels, trace=False):
    if "nc" not in _NC_CACHE:
        _NC_CACHE["nc"] = build_nc()
    nc = _NC_CACHE["nc"]
    in_maps = _host_inputs(T_F, S_F, labels)
    res = run_bass_kernel_spmd(
        nc, in_maps, core_ids=list(range(N_CORES)), trace=trace
    )
    val = np.float32(res.results[0]["out"][0, 0])
    return val, res


def kernel(T_F, S_F, labels):
    val, _ = run(T_F, S_F, labels)
    return np.array(val, dtype=np.float32)
